# revision 26
# baseline (speedup 1.0000x reference)
"""Multi-head causal attention (B=2, T=2048, E=768, H=12, HS=64) on 8 trn2 cores.

Sharding: core g handles batch b = g//4 and heads [3*(g%4), 3*(g%4)+3).
Per core, everything is computed in feature-major ("transposed") layout so no
on-device transposes are needed:
  - host ships x[b].T (e-major) in bf16, DMA'd in 512-token column slices so
    the q/k projections (looped token-group-outer) start ~5us earlier
  - qT/kT per head via weight-stationary matmuls; v in token-major layout via
    x-stationary matmuls (v rows = keys), with a ones-column appended so the
    P@v matmul also produces the softmax denominator (row l of the output)
  - scores are built transposed, S^T[s, t] = k_s . q_t, in PSUM; exp on the
    scalar engine (scale=1/8 fused) gives P^T in bf16; causal masking via a
    host-sent 128x128 triangular tile; matmuls/exp over fully-masked blocks
    are skipped by construction, and the exp calls are column-sliced so the
    masked-out left part of diagonal blocks is not computed
  - O^T = v_ext.T @ P^T accumulates in PSUM; normalization divides by the
    ones-column row via fast reciprocal + a K=1 f32r broadcast matmul
  - normalized outputs are staged per token-group: as soon as all 3 heads of
    a 512-token group are normalized, that group's [64, 3*512] block goes to
    DRAM and ONE AllGather for it fires. The first three gathers fully hide
    under the remaining attention compute; only the last ~196KB gather is
    exposed. A tiny warmup collective at kernel start absorbs the one-time
    comm-init barrier while input DMAs stream
  - each core then computes a 192-row slice of y^T = Wo_perm^T @ att^T + bo;
    token groups 0..2 project immediately after attention (PE still warm),
    a short DVE+tiny-matmul dependency chain pings the PE through the last
    gather's latency, then group 3 projects at full clock
  - queue discipline: sync = inputs + attention-internal + output DMAs,
    scalar = gather writebacks (they block on collective completion),
    gpsimd = collective triggers only
  - host reassembles: concat slices, transpose back to [B, T, E] fp32

Two program variants:
  - packed (attention_mask all ones -- the common case): no mask plumbing;
    heads 0/1 share the PE array per score matmul via row-group packing
    (K=64 each in partitions 0:64 / 64:128, concurrent in hardware)
  - general: a 65th contraction row carries the key-padding mask
    (q aug row = 1, k aug row = 8e9*(mask-1)), so masked keys get
    exp(-1e9) = 0 at zero cost; heads unpacked
"""

import numpy as np
import ml_dtypes

import concourse.bass as bass
import concourse.mybir as mybir
import concourse.tile as tile
from concourse import bacc
from concourse.bass_utils import run_bass_kernel_spmd

BF16 = ml_dtypes.bfloat16
B, T, E, H, HS = 2, 2048, 768, 12, 64
NC = 8          # cores
GS = 4          # group size (cores per batch)
HPC = 3         # heads per core
EC = E // 128   # 6 e-chunks
NT = T // 512   # 4 t-groups
NSC = T // 128  # 16 s-chunks
EO = E // GS    # 192 output columns per core
TW = HPC * 512  # staged width per t-group (3 heads x 512)
# gathered att^T row order: for head-slot j, AllGather ranks 0..3 contribute head 3r+j
PERM_HEADS = [3 * r + j for j in range(HPC) for r in range(GS)]

_CACHED = {}


def _build_program(packed):
    key = ("packed" if packed else "general")
    if key in _CACHED:
        return _CACHED[key]

    nc = bacc.Bacc("TRN2", target_bir_lowering=False, debug=False, num_devices=NC)
    dt = mybir.dt

    xT_ext = nc.dram_tensor("xt", [EC, 128, T], dt.bfloat16, kind="ExternalInput").ap()
    wqk_ext = nc.dram_tensor("wqk", [EC, 128, 384], dt.bfloat16, kind="ExternalInput").ap()
    wv_ext = nc.dram_tensor("wv", [EC, 128, 192], dt.bfloat16, kind="ExternalInput").ap()
    wo_ext = nc.dram_tensor("wo", [EC, 128, EO], dt.bfloat16, kind="ExternalInput").ap()
    bo_ext = nc.dram_tensor("bo", [EO, 1], dt.float32, kind="ExternalInput").ap()
    kaug_ext = nc.dram_tensor("kaug", [1, T], dt.bfloat16, kind="ExternalInput").ap()
    tri_ext = nc.dram_tensor("tri", [128, 128], dt.bfloat16, kind="ExternalInput").ap()
    yT_ext = nc.dram_tensor("yt", [EO, T], dt.float32, kind="ExternalOutput").ap()

    groups = [[0, 1, 2, 3], [4, 5, 6, 7]]

    with tile.TileContext(nc) as tc:
        with tc.tile_pool(name="per", bufs=1) as per, \
             tc.tile_pool(name="pp", bufs=4) as pp, \
             tc.tile_pool(name="sm", bufs=3) as sm, \
             tc.tile_pool(name="big", bufs=2, space="PSUM") as psb, \
             tc.tile_pool(name="small", bufs=4, space="PSUM") as pss, \
             tc.tile_pool(name="dram", bufs=1, space="DRAM") as dram:

            # ---- persistent SBUF ----
            xT = per.tile([128, EC * T], dt.bfloat16, tag="xT")
            wqk = per.tile([128, EC * 384], dt.bfloat16, tag="wqk")
            wv = per.tile([128, EC * 192], dt.bfloat16, tag="wv")
            wo = per.tile([128, EC * EO], dt.bfloat16, tag="wo")
            bo = per.tile([128, 2], dt.float32, tag="bo")
            tri = per.tile([128, 128], dt.bfloat16, tag="tri")
            ones_f = per.tile([1, 64], dt.float32, tag="ones_f")
            ones = per.tile([1, 64], dt.float32r, tag="ones")
            vx = per.tile([128, NSC * 195], dt.bfloat16, tag="vx")
            att = per.tile([128, EC * T], dt.bfloat16, tag="att")
            # normalized heads staged t-group-major: [64, tg*1536 + h*512]
            attn = per.tile([64, NT * TW], dt.bfloat16, tag="attn")
            if packed:
                qt01 = per.tile([128, T], dt.bfloat16, tag="qt01")
                kt01 = per.tile([128, T], dt.bfloat16, tag="kt01")
                qk2 = per.tile([128, T], dt.bfloat16, tag="qk2")
                k2s = per.tile([64, T], dt.bfloat16, tag="k2s")
                q2s = per.tile([128, T], dt.bfloat16, tag="q2s")
            else:
                qt = per.tile([128, HPC * T], dt.bfloat16, tag="qt")
                kt = per.tile([65, HPC * T], dt.bfloat16, tag="kt")

            # AllGather bounce, one pair per t-group
            ag_in = [dram.tile([64, TW], dt.bfloat16, name=f"agi{tg}", tag=f"agi{tg}")
                     for tg in range(NT)]
            ag_out = [dram.tile([GS * 64, TW], dt.bfloat16, name=f"ago{tg}", tag=f"ago{tg}")
                      for tg in range(NT)]

            # tiny collective at kernel start: absorbs the one-time comm-init
            # barrier while input DMAs stream. Input filled from a row of the
            # (host-staged) tri tensor so the trigger fires immediately.
            warm_in = dram.tile([1, 128], dt.bfloat16, name="wci", tag="wci")
            warm_out = dram.tile([GS, 128], dt.bfloat16, name="wco", tag="wco")
            nc.gpsimd.dma_start(warm_in[:], tri_ext[0:1, :])
            nc.gpsimd.collective_compute(
                "AllGather", mybir.AluOpType.bypass,
                replica_groups=groups,
                ins=[warm_in.opt()], outs=[warm_out.opt()],
            )

            # inputs: weights on the scalar queue, activations on sync, so
            # the first projection group has both after ~2us of DMA each
            for e in range(EC):
                nc.scalar.dma_start(wqk[:, e * 384:(e + 1) * 384], wqk_ext[e])
            for e in range(EC):
                nc.scalar.dma_start(wv[:, e * 192:(e + 1) * 192], wv_ext[e])
            nc.scalar.dma_start(tri[:], tri_ext[:])
            nc.scalar.dma_start(bo[0:128, 0:1], bo_ext[0:128, 0:1])
            nc.scalar.dma_start(bo[0:64, 1:2], bo_ext[128:192, 0:1])
            for tg in range(NT):
                for e in range(EC):
                    nc.sync.dma_start(
                        xT[:, e * T + 512 * tg: e * T + 512 * tg + 512],
                        xT_ext[e, :, 512 * tg: 512 * tg + 512],
                    )
            for e in range(EC):
                nc.scalar.dma_start(wo[:, e * EO:(e + 1) * EO], wo_ext[e])
            nc.vector.memset(ones_f[:], 1.0)
            nc.vector.tensor_copy(ones[:], ones_f[:])

            # ---- q/k/v projections ----
            # packed: emitted per t-group inside the attention loop (att(tg)
            # needs exactly proj(0..tg)) so the scalar engine's exp stream
            # starts ~25us earlier instead of idling through the projections.
            # chunk order [q0|q1], [k0|k1], [q2|k2] -> direct copies
            # general: chunk h = [q_h | k_h]; k rows DMA-shifted; aug rows
            def emit_proj(tg):
                qk_dst = [qt01, kt01, qk2]
                for cch in range(HPC):
                    ps = psb.tile([128, 512], dt.float32, tag="big")
                    for e in range(EC):
                        nc.tensor.matmul(
                            ps[:],
                            wqk[:, e * 384 + 128 * cch: e * 384 + 128 * cch + 128],
                            xT[:, e * T + 512 * tg: e * T + 512 * tg + 512],
                            start=(e == 0), stop=(e == EC - 1),
                        )
                    nc.vector.tensor_copy(
                        qk_dst[cch][:, 512 * tg: 512 * tg + 512], ps[:])
                    if cch == 2:
                        nc.sync.dma_start(
                            k2s[0:64, 512 * tg: 512 * tg + 512],
                            qk2[64:128, 512 * tg: 512 * tg + 512],
                        )
                        nc.sync.dma_start(
                            q2s[64:128, 512 * tg: 512 * tg + 512],
                            qk2[0:64, 512 * tg: 512 * tg + 512],
                        )
                for sc in range(4 * tg, 4 * tg + 4):
                    vp = pss.tile([128, 192], dt.float32, tag="small")
                    for e in range(EC):
                        nc.tensor.matmul(
                            vp[:],
                            xT[:, e * T + 128 * sc: e * T + 128 * sc + 128],
                            wv[:, e * 192:(e + 1) * 192],
                            start=(e == 0), stop=(e == EC - 1),
                        )
                    vdst = vx[:, 195 * sc: 195 * sc + 195]
                    nc.vector.memset(vdst, 1.0)
                    for h in range(HPC):
                        nc.vector.tensor_copy(
                            vx[:, 195 * sc + 65 * h: 195 * sc + 65 * h + 64],
                            vp[:, 64 * h: 64 * h + 64],
                        )

            if not packed:
                for h in range(HPC):
                    for tg in range(NT):
                        ps = psb.tile([128, 512], dt.float32, tag="big")
                        for e in range(EC):
                            nc.tensor.matmul(
                                ps[:],
                                wqk[:, e * 384 + 128 * h: e * 384 + 128 * h + 128],
                                xT[:, e * T + 512 * tg: e * T + 512 * tg + 512],
                                start=(e == 0), stop=(e == EC - 1),
                            )
                        dst = qt[:, h * T + 512 * tg: h * T + 512 * tg + 512]
                        nc.vector.tensor_copy(dst, ps[:])
                        nc.sync.dma_start(
                            kt[0:64, h * T + 512 * tg: h * T + 512 * tg + 512],
                            qt[64:128, h * T + 512 * tg: h * T + 512 * tg + 512],
                        )
                    nc.vector.memset(qt[64:65, h * T:(h + 1) * T], 1.0)
                    nc.sync.dma_start(kt[64:65, h * T:(h + 1) * T], kaug_ext[:])
                for sc in range(NSC):
                    vp = pss.tile([128, 192], dt.float32, tag="small")
                    for e in range(EC):
                        nc.tensor.matmul(
                            vp[:],
                            xT[:, e * T + 128 * sc: e * T + 128 * sc + 128],
                            wv[:, e * 192:(e + 1) * 192],
                            start=(e == 0), stop=(e == EC - 1),
                        )
                    vdst = vx[:, 195 * sc: 195 * sc + 195]
                    nc.vector.memset(vdst, 1.0)
                    for h in range(HPC):
                        nc.vector.tensor_copy(
                            vx[:, 195 * sc + 65 * h: 195 * sc + 65 * h + 64],
                            vp[:, 64 * h: 64 * h + 64],
                        )

            # ---- attention, software-pipelined emission ----
            # PV matmuls trail the QK/exp of the next score group; tail
            # normalizations advance one stage per group-emission point.
            pend_pv = None
            norm_q = []
            tg_done = [0] * NT

            def emit_pv(p):
                if p[0] == "AB":
                    _, tg, n_sc, (o_a, o_b), p3, sc = p
                    j = max(sc - 4 * tg, 0)
                    for hh, o_ps in ((0, o_a), (1, o_b)):
                        nc.tensor.matmul(
                            o_ps[:, 128 * j: 512],
                            vx[:, 195 * sc + 65 * hh: 195 * sc + 65 * hh + 65],
                            p3[:, 512 * hh + 128 * j: 512 * hh + 512],
                            start=(sc == 0), stop=(sc == n_sc - 1),
                        )
                    return
                h, tg, n_sc, o_ps, p3, cols = p
                for (sc, c0) in cols:
                    j = max(sc - 4 * tg, 0)
                    nc.tensor.matmul(
                        o_ps[:, 128 * j: 512],
                        vx[:, 195 * sc + 65 * h: 195 * sc + 65 * h + 65],
                        p3[:, c0 + 128 * j: c0 + 512],
                        start=(sc == 0), stop=(sc == n_sc - 1),
                    )

            def emit_norm_a(pn):
                o_ps = pn["o_ps"]
                l1 = sm.tile([1, 512], dt.float32, tag="l1")
                nc.vector.tensor_copy(l1[:], o_ps[64:65, :])
                r1 = sm.tile([1, 512], dt.float32, tag="r1")
                nc.vector.reciprocal_approx_fast(r1[:], l1[:])
                r1r = sm.tile([1, 512], dt.float32r, tag="r1r")
                nc.vector.tensor_copy(r1r[:], r1[:])
                pn["r1r"] = r1r

            def emit_gather(tg):
                # stage the finished t-group on the gpsimd queue directly
                # before its trigger (same queue -> no cross-queue stalls),
                # writebacks on sync where waiting for the collective's
                # completion cannot stall the scalar engine's exp stream
                nc.gpsimd.dma_start(
                    ag_in[tg][:], attn[0:64, tg * TW: (tg + 1) * TW])
                nc.gpsimd.collective_compute(
                    "AllGather", mybir.AluOpType.bypass,
                    replica_groups=groups,
                    ins=[ag_in[tg].opt()], outs=[ag_out[tg].opt()],
                )
                for idx in range(2 * EC):
                    e, half = divmod(idx, 2)
                    jj, rr = divmod(idx, GS)
                    nc.sync.dma_start(
                        att[64 * half: 64 * half + 64,
                            e * T + 512 * tg: e * T + 512 * tg + 512],
                        ag_out[tg][64 * rr: 64 * rr + 64, 512 * jj: 512 * jj + 512],
                    )

            def emit_norm_b(pn):
                h, tg, o_ps = pn["h"], pn["tg"], pn["o_ps"]
                rb_ps = pss.tile([64, 512], dt.float32, tag="small")
                nc.tensor.matmul(rb_ps[:], ones[:], pn["r1r"][:], start=True, stop=True)
                rb = sm.tile([64, 512], dt.float32, tag="rb")
                nc.vector.tensor_copy(rb[:], rb_ps[:])
                nc.vector.tensor_mul(
                    attn[0:64, tg * TW + 512 * h: tg * TW + 512 * h + 512],
                    o_ps[0:64, :], rb[:],
                )
                tg_done[tg] += 1
                if tg_done[tg] == HPC:
                    emit_gather(tg)

            def pump(flush_pv=True):
                nonlocal pend_pv
                if flush_pv and pend_pv is not None:
                    emit_pv(pend_pv)
                    pend_pv = None
                did_b = False
                for pn in list(norm_q):
                    if pn["stage"] == 1 and not did_b:
                        emit_norm_b(pn)
                        norm_q.remove(pn)
                        did_b = True
                    elif pn["stage"] == 0:
                        emit_norm_a(pn)
                        pn["stage"] = 1
                        break

            def act_exp(p3, s3, js):
                # js: per-512-column-slot masked-block index (0 = full slot)
                jm = min(js)
                if jm <= 0:
                    nc.scalar.activation(
                        p3[:], s3[:],
                        mybir.ActivationFunctionType.Exp, scale=0.125,
                    )
                else:
                    p3v = p3.rearrange("p (h n) -> p h n", h=2)[:, :, 128 * jm: 512]
                    s3v = s3.rearrange("p (h n) -> p h n", h=2)[:, :, 128 * jm: 512]
                    nc.scalar.activation(
                        p3v, s3v,
                        mybir.ActivationFunctionType.Exp, scale=0.125,
                    )

            if packed:
                for tg in range(NT):
                    emit_proj(tg)
                    n_sc = 4 * (tg + 1)
                    # phase A: heads 0/1, row-packed QK, one sc per group
                    o_a = pss.tile([65, 512], dt.float32, tag="small", name=f"oa{tg}")
                    o_b = pss.tile([65, 512], dt.float32, tag="small", name=f"ob{tg}")
                    for sc in range(n_sc):
                        j = max(sc - 4 * tg, 0)
                        s3 = psb.tile([128, 1024], dt.float32, tag="big")
                        for hh, pbase in ((0, 0), (1, 64)):
                            nc.tensor.matmul(
                                s3[:, 512 * hh + 128 * j: 512 * hh + 512],
                                kt01[pbase: pbase + 64, 128 * sc: 128 * sc + 128],
                                qt01[pbase: pbase + 64,
                                     512 * tg + 128 * j: 512 * tg + 512],
                                start=True, stop=True,
                            )
                        p3 = pp.tile([128, 1024], dt.bfloat16, tag="p3")
                        act_exp(p3, s3, [j, j])
                        if sc - 4 * tg >= 0:
                            for hh in range(2):
                                blk = p3[:, 512 * hh + 128 * j: 512 * hh + 128 * j + 128]
                                nc.vector.tensor_mul(blk, blk, tri[:])
                        pump()
                        pend_pv = ("AB", tg, n_sc, (o_a, o_b), p3, sc)
                    norm_q.append(dict(h=0, tg=tg, o_ps=o_a, stage=0))
                    norm_q.append(dict(h=1, tg=tg, o_ps=o_b, stage=0))
                    # phase B: head 2, two sc per group
                    o_c = pss.tile([65, 512], dt.float32, tag="small", name=f"oc{tg}")
                    scs = list(range(n_sc))
                    for g0 in range(0, n_sc, 2):
                        grp = scs[g0:g0 + 2]
                        s3 = psb.tile([128, 1024], dt.float32, tag="big")
                        for i, sc in enumerate(grp):
                            j = max(sc - 4 * tg, 0)
                            # alternate row-group halves so the two h2 score
                            # matmuls of this group run concurrently on the PE
                            if i == 0:
                                kk, qq, pb = k2s, qk2, 0
                            else:
                                kk, qq, pb = qk2, q2s, 64
                            nc.tensor.matmul(
                                s3[:, 512 * i + 128 * j: 512 * i + 512],
                                kk[pb: pb + 64, 128 * sc: 128 * sc + 128],
                                qq[pb: pb + 64, 512 * tg + 128 * j: 512 * tg + 512],
                                start=True, stop=True,
                            )
                        p3 = pp.tile([128, 1024], dt.bfloat16, tag="p3")
                        act_exp(p3, s3, [max(sc - 4 * tg, 0) for sc in grp])
                        for i, sc in enumerate(grp):
                            j = sc - 4 * tg
                            if j >= 0:
                                blk = p3[:, 512 * i + 128 * j: 512 * i + 128 * j + 128]
                                nc.vector.tensor_mul(blk, blk, tri[:])
                        pump()
                        pend_pv = (2, tg, n_sc, o_c, p3,
                                   [(sc, 512 * i) for i, sc in enumerate(grp)])
                    norm_q.append(dict(h=2, tg=tg, o_ps=o_c, stage=0))
            else:
                for h in range(HPC):
                    for tg in range(NT):
                        n_sc = 4 * (tg + 1)
                        o_ps = pss.tile([65, 512], dt.float32, tag="small")
                        scs = list(range(n_sc))
                        for g0 in range(0, n_sc, 2):
                            grp = scs[g0:g0 + 2]
                            s3 = psb.tile([128, 1024], dt.float32, tag="big")
                            for i, sc in enumerate(grp):
                                j = max(sc - 4 * tg, 0)
                                nc.tensor.matmul(
                                    s3[:, 512 * i + 128 * j: 512 * i + 512],
                                    kt[0:65, h * T + 128 * sc: h * T + 128 * sc + 128],
                                    qt[0:65, h * T + 512 * tg + 128 * j: h * T + 512 * tg + 512],
                                    start=True, stop=True,
                                )
                            p3 = pp.tile([128, 1024], dt.bfloat16, tag="p3")
                            act_exp(p3, s3, [max(sc - 4 * tg, 0) for sc in grp])
                            for i, sc in enumerate(grp):
                                j = sc - 4 * tg
                                if j >= 0:
                                    blk = p3[:, 512 * i + 128 * j: 512 * i + 128 * j + 128]
                                    nc.vector.tensor_mul(blk, blk, tri[:])
                            pump()
                            pend_pv = (h, tg, n_sc, o_ps, p3,
                                       [(sc, 512 * i) for i, sc in enumerate(grp)])
                        norm_q.append(dict(h=h, tg=tg, o_ps=o_ps, stage=0))
            # drain pipeline
            if pend_pv is not None:
                emit_pv(pend_pv)
                pend_pv = None
            while norm_q:
                pump(flush_pv=False)

            # ---- output projection ----
            def emit_yproj(tg):
                for mo in range(2):
                    m = 128 if mo == 0 else 64
                    yp = psb.tile([128, 512], dt.float32, tag="big")
                    for e in range(EC):
                        nc.tensor.matmul(
                            yp[0:m, :],
                            wo[:, e * EO + 128 * mo: e * EO + 128 * mo + m],
                            att[:, e * T + 512 * tg: e * T + 512 * tg + 512],
                            start=(e == 0), stop=(e == EC - 1),
                        )
                    y_sb = sm.tile([128, 512], dt.float32, tag="ysb")
                    nc.vector.tensor_scalar_add(y_sb[0:m, :], yp[0:m, :], bo[0:m, mo:mo + 1])
                    nc.sync.dma_start(
                        yT_ext[128 * mo: 128 * mo + m, 512 * tg: 512 * tg + 512],
                        y_sb[0:m, :],
                    )

            # groups 0..2 gathered during attention; project them while warm,
            # then the last group once its gather lands
            for tg in range(NT):
                emit_yproj(tg)

    nc.compile()
    _CACHED[key] = nc
    return nc


def _prep_inputs(x, attention_mask, Wq, Wk, Wv, Wo, bo):
    x = np.asarray(x, dtype=np.float32)
    mask = np.asarray(attention_mask)
    Wq = np.asarray(Wq, dtype=np.float32)
    Wk = np.asarray(Wk, dtype=np.float32)
    Wv = np.asarray(Wv, dtype=np.float32)
    Wo = np.asarray(Wo, dtype=np.float32)
    bo = np.asarray(bo, dtype=np.float32)

    packed = bool((np.asarray(mask) != 0).all())

    WoT = np.ascontiguousarray(Wo.T)  # [e_in, e_out]
    perm = np.concatenate([np.arange(64) + 64 * h for h in PERM_HEADS])
    WoT_perm = WoT[perm]

    tri = np.triu(np.ones((128, 128), dtype=np.float32)).astype(BF16)

    in_maps = []
    for g in range(NC):
        b, hg = g // GS, g % GS
        heads = [3 * hg + j for j in range(HPC)]
        xT = np.ascontiguousarray(x[b].T).astype(BF16).reshape(EC, 128, T)
        if packed:
            cols = [np.concatenate([Wq[heads[0]], Wq[heads[1]]], axis=1),
                    np.concatenate([Wk[heads[0]], Wk[heads[1]]], axis=1),
                    np.concatenate([Wq[heads[2]], Wk[heads[2]]], axis=1)]
        else:
            cols = [np.concatenate([Wq[h], Wk[h]], axis=1) for h in heads]
        wqk = np.concatenate(cols, axis=1).astype(BF16).reshape(EC, 128, 384)
        wv = np.concatenate([Wv[h] for h in heads], axis=1).astype(BF16).reshape(EC, 128, 192)
        eo = slice(EO * hg, EO * (hg + 1))
        wo = np.ascontiguousarray(WoT_perm[:, eo]).astype(BF16).reshape(EC, 128, EO)
        bo_sl = np.ascontiguousarray(bo[eo].reshape(EO, 1))
        kaug = (8e9 * (mask[b].astype(np.float32) - 1.0)).astype(BF16).reshape(1, T)
        in_maps.append({
            "xt": xT, "wqk": wqk, "wv": wv, "wo": wo,
            "bo": bo_sl, "kaug": kaug, "tri": tri,
        })
    return in_maps, packed


def _run(in_maps, packed, **kw):
    nc = _build_program(packed)
    return run_bass_kernel_spmd(nc, in_maps, list(range(NC)), **kw)


def _assemble(results):
    out = np.empty((B, T, E), dtype=np.float32)
    for b in range(B):
        yT = np.concatenate([results[GS * b + hg]["yt"] for hg in range(GS)], axis=0)
        out[b] = yT.T
    return out


def kernel(**inputs):
    in_maps, packed = _prep_inputs(**inputs)
    res = _run(in_maps, packed)
    return _assemble(res.results)


# revision 30
# speedup vs baseline: 1.0606x; 1.0606x over previous
"""Multi-head causal attention (B=2, T=2048, E=768, H=12, HS=64) on 8 trn2 cores.

Sharding: core g handles batch b = g//4 and heads [3*(g%4), 3*(g%4)+3).
Per core, everything is computed in feature-major ("transposed") layout so no
on-device transposes are needed:
  - host ships x[b].T (e-major) in bf16, DMA'd in 512-token column slices so
    the q/k projections (looped token-group-outer) start ~5us earlier
  - qT/kT per head via weight-stationary matmuls; v in token-major layout via
    x-stationary matmuls (v rows = keys), with a ones-column appended so the
    P@v matmul also produces the softmax denominator (row l of the output)
  - scores are built transposed, S^T[s, t] = k_s . q_t, in PSUM; exp on the
    scalar engine (scale=1/8 fused) gives P^T in bf16; causal masking via a
    host-sent 128x128 triangular tile; matmuls/exp over fully-masked blocks
    are skipped by construction, and the exp calls are column-sliced so the
    masked-out left part of diagonal blocks is not computed
  - O^T = v_ext.T @ P^T accumulates in PSUM; normalization divides by the
    ones-column row via fast reciprocal + a K=1 f32r broadcast matmul
  - normalized outputs are staged per token-group: as soon as all 3 heads of
    a 512-token group are normalized, that group's [64, 3*512] block goes to
    DRAM and ONE AllGather for it fires. The first three gathers fully hide
    under the remaining attention compute; only the last ~196KB gather is
    exposed. A tiny warmup collective at kernel start absorbs the one-time
    comm-init barrier while input DMAs stream
  - each core then computes a 192-row slice of y^T = Wo_perm^T @ att^T + bo;
    token groups 0..2 project immediately after attention (PE still warm),
    a short DVE+tiny-matmul dependency chain pings the PE through the last
    gather's latency, then group 3 projects at full clock
  - queue discipline: sync = inputs + attention-internal + output DMAs,
    scalar = gather writebacks (they block on collective completion),
    gpsimd = collective triggers only
  - host reassembles: concat slices, transpose back to [B, T, E] fp32

Two program variants:
  - packed (attention_mask all ones -- the common case): no mask plumbing;
    heads 0/1 share the PE array per score matmul via row-group packing
    (K=64 each in partitions 0:64 / 64:128, concurrent in hardware)
  - general: a 65th contraction row carries the key-padding mask
    (q aug row = 1, k aug row = 8e9*(mask-1)), so masked keys get
    exp(-1e9) = 0 at zero cost; heads unpacked
"""

import numpy as np
import ml_dtypes

import concourse.bass as bass
import concourse.mybir as mybir
import concourse.tile as tile
from concourse import bacc
from concourse.bass_utils import run_bass_kernel_spmd

BF16 = ml_dtypes.bfloat16
B, T, E, H, HS = 2, 2048, 768, 12, 64
NC = 8          # cores
GS = 4          # group size (cores per batch)
HPC = 3         # heads per core
EC = E // 128   # 6 e-chunks
NT = T // 512   # 4 t-groups
NSC = T // 128  # 16 s-chunks
EO = E // GS    # 192 output columns per core
TW = HPC * 512  # staged width per t-group (3 heads x 512)
# gathered att^T row order: for head-slot j, AllGather ranks 0..3 contribute head 3r+j
PERM_HEADS = [3 * r + j for j in range(HPC) for r in range(GS)]

_CACHED = {}


def _build_program(packed):
    key = ("packed" if packed else "general")
    if key in _CACHED:
        return _CACHED[key]

    nc = bacc.Bacc("TRN2", target_bir_lowering=False, debug=False, num_devices=NC)
    dt = mybir.dt

    xT_ext = nc.dram_tensor("xt", [EC, 128, T], dt.bfloat16, kind="ExternalInput").ap()
    wqk_ext = nc.dram_tensor("wqk", [EC, 128, 384], dt.bfloat16, kind="ExternalInput").ap()
    wv_ext = nc.dram_tensor("wv", [EC, 128, 192], dt.bfloat16, kind="ExternalInput").ap()
    wo_ext = nc.dram_tensor("wo", [EC, 128, EO], dt.bfloat16, kind="ExternalInput").ap()
    bo_ext = nc.dram_tensor("bo", [EO, 1], dt.float32, kind="ExternalInput").ap()
    kaug_ext = nc.dram_tensor("kaug", [1, T], dt.bfloat16, kind="ExternalInput").ap()
    tri_ext = nc.dram_tensor("tri", [128, 128], dt.bfloat16, kind="ExternalInput").ap()
    yT_ext = nc.dram_tensor("yt", [EO, T], dt.float32, kind="ExternalOutput").ap()

    groups = [[0, 1, 2, 3], [4, 5, 6, 7]]

    with tile.TileContext(nc) as tc:
        with tc.tile_pool(name="per", bufs=1) as per, \
             tc.tile_pool(name="pp", bufs=4) as pp, \
             tc.tile_pool(name="sm", bufs=3) as sm, \
             tc.tile_pool(name="big", bufs=2, space="PSUM") as psb, \
             tc.tile_pool(name="small", bufs=4, space="PSUM") as pss, \
             tc.tile_pool(name="dram", bufs=1, space="DRAM") as dram:

            # ---- persistent SBUF ----
            xT = per.tile([128, EC * T], dt.bfloat16, tag="xT")
            wqk = per.tile([128, EC * 384], dt.bfloat16, tag="wqk")
            wv = per.tile([128, EC * 192], dt.bfloat16, tag="wv")
            wo = per.tile([128, EC * EO], dt.bfloat16, tag="wo")
            bo = per.tile([128, 2], dt.float32, tag="bo")
            tri = per.tile([128, 128], dt.bfloat16, tag="tri")
            ones_f = per.tile([1, 64], dt.float32, tag="ones_f")
            ones = per.tile([1, 64], dt.float32r, tag="ones")
            vx = per.tile([128, NSC * 195], dt.bfloat16, tag="vx")
            att = per.tile([128, EC * T], dt.bfloat16, tag="att")
            # normalized heads staged t-group-major: [64, tg*1536 + h*512]
            attn = per.tile([64, NT * TW], dt.bfloat16, tag="attn")
            if packed:
                qt01 = per.tile([128, T], dt.bfloat16, tag="qt01")
                kt01 = per.tile([128, T], dt.bfloat16, tag="kt01")
                qk2 = per.tile([128, T], dt.bfloat16, tag="qk2")
                k2s = per.tile([64, T], dt.bfloat16, tag="k2s")
                q2s = per.tile([128, T], dt.bfloat16, tag="q2s")
            else:
                qt = per.tile([128, HPC * T], dt.bfloat16, tag="qt")
                kt = per.tile([65, HPC * T], dt.bfloat16, tag="kt")

            # AllGather bounce, one pair per t-group
            ag_in = [dram.tile([64, TW], dt.bfloat16, name=f"agi{tg}", tag=f"agi{tg}")
                     for tg in range(NT)]
            ag_out = [dram.tile([GS * 64, TW], dt.bfloat16, name=f"ago{tg}", tag=f"ago{tg}")
                      for tg in range(NT)]

            # tiny collective at kernel start: absorbs the one-time comm-init
            # barrier while input DMAs stream. Input filled from a row of the
            # (host-staged) tri tensor so the trigger fires immediately.
            warm_in = dram.tile([1, 128], dt.bfloat16, name="wci", tag="wci")
            warm_out = dram.tile([GS, 128], dt.bfloat16, name="wco", tag="wco")
            nc.gpsimd.dma_start(warm_in[:], tri_ext[0:1, :])
            nc.gpsimd.collective_compute(
                "AllGather", mybir.AluOpType.bypass,
                replica_groups=groups,
                ins=[warm_in.opt()], outs=[warm_out.opt()],
            )

            # inputs: wqk ahead of the x slices on sync (the x slices for
            # each t-group are emitted inside emit_proj so per-group DMAs
            # land just in time); the small weights ride the scalar queue
            # and finish well before the first exp needs that queue
            for e in range(EC):
                nc.sync.dma_start(wqk[:, e * 384:(e + 1) * 384], wqk_ext[e])
            for e in range(EC):
                nc.scalar.dma_start(wv[:, e * 192:(e + 1) * 192], wv_ext[e])
            nc.scalar.dma_start(tri[:], tri_ext[:])
            nc.scalar.dma_start(bo[0:128, 0:1], bo_ext[0:128, 0:1])
            nc.scalar.dma_start(bo[0:64, 1:2], bo_ext[128:192, 0:1])
            for e in range(EC):
                nc.scalar.dma_start(wo[:, e * EO:(e + 1) * EO], wo_ext[e])
            nc.vector.memset(ones_f[:], 1.0)
            nc.vector.tensor_copy(ones[:], ones_f[:])

            # ---- q/k/v projections ----
            # packed: emitted per t-group inside the attention loop (att(tg)
            # needs exactly proj(0..tg)) so the scalar engine's exp stream
            # starts ~25us earlier instead of idling through the projections.
            # chunk order [q0|q1], [k0|k1], [q2|k2] -> direct copies
            # general: chunk h = [q_h | k_h]; k rows DMA-shifted; aug rows
            def emit_proj(tg):
                for e in range(EC):
                    nc.sync.dma_start(
                        xT[:, e * T + 512 * tg: e * T + 512 * tg + 512],
                        xT_ext[e, :, 512 * tg: 512 * tg + 512],
                    )
                qk_dst = [qt01, kt01, qk2]
                for cch in range(HPC):
                    ps = psb.tile([128, 512], dt.float32, tag="big")
                    for e in range(EC):
                        nc.tensor.matmul(
                            ps[:],
                            wqk[:, e * 384 + 128 * cch: e * 384 + 128 * cch + 128],
                            xT[:, e * T + 512 * tg: e * T + 512 * tg + 512],
                            start=(e == 0), stop=(e == EC - 1),
                        )
                    nc.vector.tensor_copy(
                        qk_dst[cch][:, 512 * tg: 512 * tg + 512], ps[:])
                    if cch == 2:
                        nc.sync.dma_start(
                            k2s[0:64, 512 * tg: 512 * tg + 512],
                            qk2[64:128, 512 * tg: 512 * tg + 512],
                        )
                        nc.sync.dma_start(
                            q2s[64:128, 512 * tg: 512 * tg + 512],
                            qk2[0:64, 512 * tg: 512 * tg + 512],
                        )
                for sc in range(4 * tg, 4 * tg + 4):
                    vp = pss.tile([128, 192], dt.float32, tag="small")
                    for e in range(EC):
                        nc.tensor.matmul(
                            vp[:],
                            xT[:, e * T + 128 * sc: e * T + 128 * sc + 128],
                            wv[:, e * 192:(e + 1) * 192],
                            start=(e == 0), stop=(e == EC - 1),
                        )
                    vdst = vx[:, 195 * sc: 195 * sc + 195]
                    nc.vector.memset(vdst, 1.0)
                    for h in range(HPC):
                        nc.vector.tensor_copy(
                            vx[:, 195 * sc + 65 * h: 195 * sc + 65 * h + 64],
                            vp[:, 64 * h: 64 * h + 64],
                        )

            if not packed:
                for tg in range(NT):
                    for e in range(EC):
                        nc.sync.dma_start(
                            xT[:, e * T + 512 * tg: e * T + 512 * tg + 512],
                            xT_ext[e, :, 512 * tg: 512 * tg + 512],
                        )
                for h in range(HPC):
                    for tg in range(NT):
                        ps = psb.tile([128, 512], dt.float32, tag="big")
                        for e in range(EC):
                            nc.tensor.matmul(
                                ps[:],
                                wqk[:, e * 384 + 128 * h: e * 384 + 128 * h + 128],
                                xT[:, e * T + 512 * tg: e * T + 512 * tg + 512],
                                start=(e == 0), stop=(e == EC - 1),
                            )
                        dst = qt[:, h * T + 512 * tg: h * T + 512 * tg + 512]
                        nc.vector.tensor_copy(dst, ps[:])
                        nc.sync.dma_start(
                            kt[0:64, h * T + 512 * tg: h * T + 512 * tg + 512],
                            qt[64:128, h * T + 512 * tg: h * T + 512 * tg + 512],
                        )
                    nc.vector.memset(qt[64:65, h * T:(h + 1) * T], 1.0)
                    nc.sync.dma_start(kt[64:65, h * T:(h + 1) * T], kaug_ext[:])
                for sc in range(NSC):
                    vp = pss.tile([128, 192], dt.float32, tag="small")
                    for e in range(EC):
                        nc.tensor.matmul(
                            vp[:],
                            xT[:, e * T + 128 * sc: e * T + 128 * sc + 128],
                            wv[:, e * 192:(e + 1) * 192],
                            start=(e == 0), stop=(e == EC - 1),
                        )
                    vdst = vx[:, 195 * sc: 195 * sc + 195]
                    nc.vector.memset(vdst, 1.0)
                    for h in range(HPC):
                        nc.vector.tensor_copy(
                            vx[:, 195 * sc + 65 * h: 195 * sc + 65 * h + 64],
                            vp[:, 64 * h: 64 * h + 64],
                        )

            # ---- attention, software-pipelined emission ----
            # PV matmuls trail the QK/exp of the next score group; tail
            # normalizations advance one stage per group-emission point.
            pend_pv = None
            norm_q = []
            tg_done = [0] * NT

            def emit_pv(p):
                if p[0] == "AB":
                    _, tg, n_sc, (o_a, o_b), p3, sc = p
                    j = max(sc - 4 * tg, 0)
                    for hh, o_ps in ((0, o_a), (1, o_b)):
                        nc.tensor.matmul(
                            o_ps[:, 128 * j: 512],
                            vx[:, 195 * sc + 65 * hh: 195 * sc + 65 * hh + 65],
                            p3[:, 512 * hh + 128 * j: 512 * hh + 512],
                            start=(sc == 0), stop=(sc == n_sc - 1),
                        )
                    return
                h, tg, n_sc, o_ps, p3, cols = p
                for (sc, c0) in cols:
                    j = max(sc - 4 * tg, 0)
                    nc.tensor.matmul(
                        o_ps[:, 128 * j: 512],
                        vx[:, 195 * sc + 65 * h: 195 * sc + 65 * h + 65],
                        p3[:, c0 + 128 * j: c0 + 512],
                        start=(sc == 0), stop=(sc == n_sc - 1),
                    )

            def emit_norm_a(pn):
                o_ps = pn["o_ps"]
                l1 = sm.tile([1, 512], dt.float32, tag="l1")
                nc.vector.tensor_copy(l1[:], o_ps[64:65, :])
                r1 = sm.tile([1, 512], dt.float32, tag="r1")
                nc.vector.reciprocal_approx_fast(r1[:], l1[:])
                r1r = sm.tile([1, 512], dt.float32r, tag="r1r")
                nc.vector.tensor_copy(r1r[:], r1[:])
                pn["r1r"] = r1r

            def emit_gather(tg):
                # stage the finished t-group via the scalar queue (costs the
                # exp stream <1us mid-attention, nothing at the end, and
                # avoids getting stuck behind a trigger's completion-wait on
                # gpsimd); writebacks on sync where waiting for the
                # collective's completion cannot stall anything hot
                nc.scalar.dma_start(
                    ag_in[tg][:], attn[0:64, tg * TW: (tg + 1) * TW])
                nc.gpsimd.collective_compute(
                    "AllGather", mybir.AluOpType.bypass,
                    replica_groups=groups,
                    ins=[ag_in[tg].opt()], outs=[ag_out[tg].opt()],
                )
                for idx in range(2 * EC):
                    e, half = divmod(idx, 2)
                    jj, rr = divmod(idx, GS)
                    nc.sync.dma_start(
                        att[64 * half: 64 * half + 64,
                            e * T + 512 * tg: e * T + 512 * tg + 512],
                        ag_out[tg][64 * rr: 64 * rr + 64, 512 * jj: 512 * jj + 512],
                    )

            def emit_norm_b(pn):
                h, tg, o_ps = pn["h"], pn["tg"], pn["o_ps"]
                rb_ps = pss.tile([64, 512], dt.float32, tag="small")
                nc.tensor.matmul(rb_ps[:], ones[:], pn["r1r"][:], start=True, stop=True)
                rb = sm.tile([64, 512], dt.float32, tag="rb")
                nc.vector.tensor_copy(rb[:], rb_ps[:])
                nc.vector.tensor_mul(
                    attn[0:64, tg * TW + 512 * h: tg * TW + 512 * h + 512],
                    o_ps[0:64, :], rb[:],
                )
                tg_done[tg] += 1
                if tg_done[tg] == HPC:
                    emit_gather(tg)

            def pump(flush_pv=True):
                nonlocal pend_pv
                if flush_pv and pend_pv is not None:
                    emit_pv(pend_pv)
                    pend_pv = None
                did_b = False
                for pn in list(norm_q):
                    if pn["stage"] == 1 and not did_b:
                        emit_norm_b(pn)
                        norm_q.remove(pn)
                        did_b = True
                    elif pn["stage"] == 0:
                        emit_norm_a(pn)
                        pn["stage"] = 1
                        break

            def act_exp(p3, s3, js):
                # js: per-512-column-slot masked-block index (0 = full slot)
                jm = min(js)
                if jm <= 0:
                    nc.scalar.activation(
                        p3[:], s3[:],
                        mybir.ActivationFunctionType.Exp, scale=0.125,
                    )
                else:
                    p3v = p3.rearrange("p (h n) -> p h n", h=2)[:, :, 128 * jm: 512]
                    s3v = s3.rearrange("p (h n) -> p h n", h=2)[:, :, 128 * jm: 512]
                    nc.scalar.activation(
                        p3v, s3v,
                        mybir.ActivationFunctionType.Exp, scale=0.125,
                    )

            if packed:
                for tg in range(NT):
                    emit_proj(tg)
                    n_sc = 4 * (tg + 1)
                    # phase A: heads 0/1, row-packed QK, one sc per group
                    o_a = pss.tile([65, 512], dt.float32, tag="small", name=f"oa{tg}")
                    o_b = pss.tile([65, 512], dt.float32, tag="small", name=f"ob{tg}")
                    for sc in range(n_sc):
                        j = max(sc - 4 * tg, 0)
                        s3 = psb.tile([128, 1024], dt.float32, tag="big")
                        for hh, pbase in ((0, 0), (1, 64)):
                            nc.tensor.matmul(
                                s3[:, 512 * hh + 128 * j: 512 * hh + 512],
                                kt01[pbase: pbase + 64, 128 * sc: 128 * sc + 128],
                                qt01[pbase: pbase + 64,
                                     512 * tg + 128 * j: 512 * tg + 512],
                                start=True, stop=True,
                            )
                        p3 = pp.tile([128, 1024], dt.bfloat16, tag="p3")
                        act_exp(p3, s3, [j, j])
                        if sc - 4 * tg >= 0:
                            for hh in range(2):
                                blk = p3[:, 512 * hh + 128 * j: 512 * hh + 128 * j + 128]
                                nc.vector.tensor_mul(blk, blk, tri[:])
                        pump()
                        pend_pv = ("AB", tg, n_sc, (o_a, o_b), p3, sc)
                    norm_q.append(dict(h=0, tg=tg, o_ps=o_a, stage=0))
                    norm_q.append(dict(h=1, tg=tg, o_ps=o_b, stage=0))
                    # phase B: head 2, two sc per group
                    o_c = pss.tile([65, 512], dt.float32, tag="small", name=f"oc{tg}")
                    scs = list(range(n_sc))
                    for g0 in range(0, n_sc, 2):
                        grp = scs[g0:g0 + 2]
                        s3 = psb.tile([128, 1024], dt.float32, tag="big")
                        for i, sc in enumerate(grp):
                            j = max(sc - 4 * tg, 0)
                            # alternate row-group halves so the two h2 score
                            # matmuls of this group run concurrently on the PE
                            if i == 0:
                                kk, qq, pb = k2s, qk2, 0
                            else:
                                kk, qq, pb = qk2, q2s, 64
                            nc.tensor.matmul(
                                s3[:, 512 * i + 128 * j: 512 * i + 512],
                                kk[pb: pb + 64, 128 * sc: 128 * sc + 128],
                                qq[pb: pb + 64, 512 * tg + 128 * j: 512 * tg + 512],
                                start=True, stop=True,
                            )
                        p3 = pp.tile([128, 1024], dt.bfloat16, tag="p3")
                        act_exp(p3, s3, [max(sc - 4 * tg, 0) for sc in grp])
                        for i, sc in enumerate(grp):
                            j = sc - 4 * tg
                            if j >= 0:
                                blk = p3[:, 512 * i + 128 * j: 512 * i + 128 * j + 128]
                                nc.vector.tensor_mul(blk, blk, tri[:])
                        pump()
                        pend_pv = (2, tg, n_sc, o_c, p3,
                                   [(sc, 512 * i) for i, sc in enumerate(grp)])
                    norm_q.append(dict(h=2, tg=tg, o_ps=o_c, stage=0))
            else:
                for h in range(HPC):
                    for tg in range(NT):
                        n_sc = 4 * (tg + 1)
                        o_ps = pss.tile([65, 512], dt.float32, tag="small")
                        scs = list(range(n_sc))
                        for g0 in range(0, n_sc, 2):
                            grp = scs[g0:g0 + 2]
                            s3 = psb.tile([128, 1024], dt.float32, tag="big")
                            for i, sc in enumerate(grp):
                                j = max(sc - 4 * tg, 0)
                                nc.tensor.matmul(
                                    s3[:, 512 * i + 128 * j: 512 * i + 512],
                                    kt[0:65, h * T + 128 * sc: h * T + 128 * sc + 128],
                                    qt[0:65, h * T + 512 * tg + 128 * j: h * T + 512 * tg + 512],
                                    start=True, stop=True,
                                )
                            p3 = pp.tile([128, 1024], dt.bfloat16, tag="p3")
                            act_exp(p3, s3, [max(sc - 4 * tg, 0) for sc in grp])
                            for i, sc in enumerate(grp):
                                j = sc - 4 * tg
                                if j >= 0:
                                    blk = p3[:, 512 * i + 128 * j: 512 * i + 128 * j + 128]
                                    nc.vector.tensor_mul(blk, blk, tri[:])
                            pump()
                            pend_pv = (h, tg, n_sc, o_ps, p3,
                                       [(sc, 512 * i) for i, sc in enumerate(grp)])
                        norm_q.append(dict(h=h, tg=tg, o_ps=o_ps, stage=0))
            # drain pipeline
            if pend_pv is not None:
                emit_pv(pend_pv)
                pend_pv = None
            while norm_q:
                pump(flush_pv=False)

            # ---- output projection ----
            def emit_yproj(tg):
                for mo in range(2):
                    m = 128 if mo == 0 else 64
                    yp = psb.tile([128, 512], dt.float32, tag="big")
                    for e in range(EC):
                        nc.tensor.matmul(
                            yp[0:m, :],
                            wo[:, e * EO + 128 * mo: e * EO + 128 * mo + m],
                            att[:, e * T + 512 * tg: e * T + 512 * tg + 512],
                            start=(e == 0), stop=(e == EC - 1),
                        )
                    y_sb = sm.tile([128, 512], dt.float32, tag="ysb")
                    nc.vector.tensor_scalar_add(y_sb[0:m, :], yp[0:m, :], bo[0:m, mo:mo + 1])
                    nc.sync.dma_start(
                        yT_ext[128 * mo: 128 * mo + m, 512 * tg: 512 * tg + 512],
                        y_sb[0:m, :],
                    )

            # groups 0..2 gathered during attention; project them while warm,
            # then the last group once its gather lands
            for tg in range(NT):
                emit_yproj(tg)

    nc.compile()
    _CACHED[key] = nc
    return nc


def _prep_inputs(x, attention_mask, Wq, Wk, Wv, Wo, bo):
    x = np.asarray(x, dtype=np.float32)
    mask = np.asarray(attention_mask)
    Wq = np.asarray(Wq, dtype=np.float32)
    Wk = np.asarray(Wk, dtype=np.float32)
    Wv = np.asarray(Wv, dtype=np.float32)
    Wo = np.asarray(Wo, dtype=np.float32)
    bo = np.asarray(bo, dtype=np.float32)

    packed = bool((np.asarray(mask) != 0).all())

    WoT = np.ascontiguousarray(Wo.T)  # [e_in, e_out]
    perm = np.concatenate([np.arange(64) + 64 * h for h in PERM_HEADS])
    WoT_perm = WoT[perm]

    tri = np.triu(np.ones((128, 128), dtype=np.float32)).astype(BF16)

    in_maps = []
    for g in range(NC):
        b, hg = g // GS, g % GS
        heads = [3 * hg + j for j in range(HPC)]
        xT = np.ascontiguousarray(x[b].T).astype(BF16).reshape(EC, 128, T)
        if packed:
            cols = [np.concatenate([Wq[heads[0]], Wq[heads[1]]], axis=1),
                    np.concatenate([Wk[heads[0]], Wk[heads[1]]], axis=1),
                    np.concatenate([Wq[heads[2]], Wk[heads[2]]], axis=1)]
        else:
            cols = [np.concatenate([Wq[h], Wk[h]], axis=1) for h in heads]
        wqk = np.concatenate(cols, axis=1).astype(BF16).reshape(EC, 128, 384)
        wv = np.concatenate([Wv[h] for h in heads], axis=1).astype(BF16).reshape(EC, 128, 192)
        eo = slice(EO * hg, EO * (hg + 1))
        wo = np.ascontiguousarray(WoT_perm[:, eo]).astype(BF16).reshape(EC, 128, EO)
        bo_sl = np.ascontiguousarray(bo[eo].reshape(EO, 1))
        kaug = (8e9 * (mask[b].astype(np.float32) - 1.0)).astype(BF16).reshape(1, T)
        in_maps.append({
            "xt": xT, "wqk": wqk, "wv": wv, "wo": wo,
            "bo": bo_sl, "kaug": kaug, "tri": tri,
        })
    return in_maps, packed


def _run(in_maps, packed, **kw):
    nc = _build_program(packed)
    return run_bass_kernel_spmd(nc, in_maps, list(range(NC)), **kw)


def _assemble(results):
    out = np.empty((B, T, E), dtype=np.float32)
    for b in range(B):
        yT = np.concatenate([results[GS * b + hg]["yt"] for hg in range(GS)], axis=0)
        out[b] = yT.T
    return out


def kernel(**inputs):
    in_maps, packed = _prep_inputs(**inputs)
    res = _run(in_maps, packed)
    return _assemble(res.results)
